# revision 5
# baseline (speedup 1.0000x reference)
"""DeepSeek-style MoE block on 8 Trainium2 NeuronCores.

Sharding strategy (expert-parallel, per the problem's sharding hint):
  - The 8 routed experts are sharded one-per-core: core e holds We1[e]/We3[e]/We2[e].
  - Token dispatch (gate -> top-2 -> gather per expert) is computed on host;
    each core receives its expert's gathered token batch, padded to a common
    capacity C, in transposed [d_model, C] layout so no on-device transposes
    are needed (activations stay [feature, token] throughout).
  - The shared expert is data-parallel: core e processes tokens [e*512,(e+1)*512).
  - Matmuls run in bf16 with fp32 PSUM accumulation; outputs are fp32.
  - Host scatter-adds the per-expert outputs (already scaled by the normalized
    gate weight / 3 on device) and the shared outputs (scaled by 1/3 on device).
"""

import os

import numpy as np
import ml_dtypes

import concourse.bass as bass
import concourse.mybir as mybir
import concourse.tile as tile_mod
from concourse.bass_utils import run_bass_kernel_spmd
from concourse.vector_clock import ScopedClock

_BF16 = ml_dtypes.bfloat16
P = 128
D_MODEL = 1024
FF_DIM = 2048
N_ROUTED = 8
TOP_K = 2
N_CORES = 8
T_TOKENS = 4096           # 2 * 2048
TS = T_TOKENS // N_CORES  # shared-expert tokens per core
KO1 = D_MODEL // P        # 8  k-chunks for d_model contraction
KO2 = FF_DIM // P         # 16 k-chunks for ff contraction
NT = 512                  # token tile (matmul moving free dim)

LAST_EXEC_NS = None       # set after each kernel() call when profiling


def _split_drain_and_barrier(self, tick_clock, wait_clock):
    """Tile's kernel-tail drain carries one sem-wait per touched engine/queue;
    this walrus build rejects >2 sync waits per instruction. Split the waits
    into single-wait sync nops ahead of the drain (semantically identical:
    the SP stream blocks on each wait in sequence before drain+barrier)."""
    nc = self.nc
    probe = nc.sync.nop(nofuse=True)
    wait_clock.add_sem_waits(probe.ins, ScopedClock({None: tick_clock.global_clock}))
    waits = list(probe.ins.sync_info.on_wait)
    if len(waits) > 1:
        probe.ins.sync_info.on_wait[:] = waits[:1]
        SyncInfo = type(probe.ins.sync_info)
        for w in waits[1:]:
            n2 = nc.sync.nop(nofuse=True)
            n2.ins.sync_info = SyncInfo(on_update=[], on_wait=[w])
    nc.sync.drain()
    nc.all_engine_barrier()
    popped = nc._tile_sem_poison_stack.pop()
    assert popped is self._sem_poison
    nc.clear_and_free_semaphores(list(self.sems.allocated().values()))
    nc.all_engine_barrier()


_MAX_WAITS = 1  # this walrus build rejects multiple sync waits on one instruction


def _split_body_waits(self, postordered_blocks):
    """Before lowering the scheduled instruction lists into basic blocks,
    hoist excess sem-waits (>_MAX_WAITS) of any instruction onto same-engine
    nops inserted immediately before it. Same-engine order is preserved, so
    semantics are identical."""
    nc = self.nc
    for insts in postordered_blocks.values():
        needs_fix = any(
            getattr(ins, "sync_info", None) is not None
            and len(ins.sync_info.on_wait) > _MAX_WAITS
            and getattr(ins, "engine", None) is not None
            for ins in insts
        )
        if not needs_fix:
            continue
        out = []
        for ins in insts:
            si = getattr(ins, "sync_info", None)
            if (si is not None and len(si.on_wait) > _MAX_WAITS
                    and getattr(ins, "engine", None) is not None):
                waits = list(si.on_wait)
                excess, keep = waits[:-_MAX_WAITS], waits[-_MAX_WAITS:]
                si.on_wait[:] = keep
                for i in range(0, len(excess), _MAX_WAITS):
                    out.append(mybir.InstNoOp(
                        name=nc.get_next_instruction_name(),
                        engine=ins.engine,
                        bass_nofuse=True,
                        sync_info=mybir.SyncInfo(
                            on_wait=list(excess[i:i + _MAX_WAITS]), on_update=[]),
                    ))
            out.append(ins)
        insts[:] = out
    return tile_mod.TileContext._orig_lower_ordered_insts(self, postordered_blocks)


def _install_drain_fix():
    if getattr(tile_mod.TileContext, "_drain_fix_installed", False):
        return
    tile_mod.TileContext._drain_and_barrier = _split_drain_and_barrier
    tile_mod.TileContext._orig_lower_ordered_insts = (
        tile_mod.TileContext._lower_ordered_insts)
    tile_mod.TileContext._lower_ordered_insts = _split_body_waits
    tile_mod.TileContext._drain_fix_installed = True


def _install_profiling_shims():
    """Best-effort NTFF profiling under axon: the container's `antenv` lacks
    `axon_hooks`, so build it and register the ctypes hook from trn_agent_boot.
    Also neuter the S3 artifact upload (no credentials here)."""
    import sys
    import types

    import concourse.bass_utils as bu
    bu.upload_artifacts = lambda tmpdir: tmpdir

    try:
        import antenv.axon_hooks  # noqa: F401
        return True
    except ImportError:
        pass
    try:
        from trn_agent_boot.trn_boot import _ntff_profile_via_ctypes
        hook = _ntff_profile_via_ctypes("/opt/axon/libaxon_pjrt.so")
    except Exception:
        return False
    if hook is None:
        return False
    m = types.ModuleType("antenv.axon_hooks")
    _state = {"h": hook}
    m.get_axon_ntff_profile_hook = lambda: _state["h"]
    m.set_axon_ntff_profile_hook = lambda h: _state.__setitem__("h", h)
    sys.modules["antenv.axon_hooks"] = m
    import antenv
    antenv.axon_hooks = m
    return True


def _token_tiles(c):
    tiles, t0 = [], 0
    while t0 < c:
        tn = min(NT, c - t0)
        tiles.append((t0, tn))
        t0 += tn
    return tiles


def _build_program(C):
    """One SPMD program: expert FFN over C gathered tokens + shared FFN over TS
    tokens. All activations live in [feature, token] (transposed) layout."""
    _install_drain_fix()
    nc = bass.Bass("TRN2")
    f32, bf16 = mybir.dt.float32, mybir.dt.bfloat16
    SILU = mybir.ActivationFunctionType.Silu

    xg = nc.dram_tensor("xg", [D_MODEL, C], bf16, kind="ExternalInput")
    wgt = nc.dram_tensor("wgt", [P, C], f32, kind="ExternalInput")
    xs = nc.dram_tensor("xs", [D_MODEL, TS], bf16, kind="ExternalInput")
    we1 = nc.dram_tensor("we1", [D_MODEL, FF_DIM], bf16, kind="ExternalInput")
    we3 = nc.dram_tensor("we3", [D_MODEL, FF_DIM], bf16, kind="ExternalInput")
    we2 = nc.dram_tensor("we2", [FF_DIM, D_MODEL], bf16, kind="ExternalInput")
    ws1 = nc.dram_tensor("ws1", [D_MODEL, FF_DIM], bf16, kind="ExternalInput")
    ws3 = nc.dram_tensor("ws3", [D_MODEL, FF_DIM], bf16, kind="ExternalInput")
    ws2 = nc.dram_tensor("ws2", [FF_DIM, D_MODEL], bf16, kind="ExternalInput")
    ye = nc.dram_tensor("ye", [D_MODEL, C], f32, kind="ExternalOutput")
    ys = nc.dram_tensor("ys", [D_MODEL, TS], f32, kind="ExternalOutput")

    with tile_mod.TileContext(nc) as tc:
        with (
            tc.tile_pool(name="wpool", bufs=1) as wpool,
            tc.tile_pool(name="xpool", bufs=2) as xpool,
            tc.tile_pool(name="hpool", bufs=2) as hpool,
            tc.tile_pool(name="spool", bufs=3) as spool,
            tc.tile_pool(name="ypool", bufs=3) as ypool,
            tc.tile_pool(name="cpool", bufs=1) as cpool,
            tc.tile_pool(name="pspool", bufs=2, space="PSUM") as pspool,
        ):
            wb = cpool.tile([P, C], f32, tag="wb")
            nc.sync.dma_start(wb[:], wgt[:, :])

            def ffn(xT, w1d, w3d, w2d, outd, Ct, scale_tile):
                w1 = wpool.tile([P, KO1, FF_DIM], bf16, tag="w1")
                nc.sync.dma_start(w1[:], w1d[:, :].rearrange("(ko p) f -> p ko f", p=P))
                w3 = wpool.tile([P, KO1, FF_DIM], bf16, tag="w3")
                nc.sync.dma_start(w3[:], w3d[:, :].rearrange("(ko p) f -> p ko f", p=P))
                w2 = wpool.tile([P, KO2, D_MODEL], bf16, tag="w2")
                nc.sync.dma_start(w2[:], w2d[:, :].rearrange("(ko p) f -> p ko f", p=P))
                xr = xT[:, :].rearrange("(ko p) t -> p ko t", p=P)
                outr = outd[:, :].rearrange("(ko p) t -> p ko t", p=P)
                for (t0, tn) in _token_tiles(Ct):
                    xt = xpool.tile([P, KO1, NT], bf16, tag="xt")
                    nc.sync.dma_start(xt[:, :, :tn], xr[:, :, t0:t0 + tn])
                    h = hpool.tile([P, KO2, NT], bf16, tag="h")
                    for f in range(KO2):
                        p1 = pspool.tile([P, NT], f32, tag="p1")
                        p3 = pspool.tile([P, NT], f32, tag="p3")
                        for k in range(KO1):
                            nc.tensor.matmul(
                                p1[:, :tn], w1[:, k, f * P:(f + 1) * P],
                                xt[:, k, :tn], start=(k == 0), stop=(k == KO1 - 1))
                        for k in range(KO1):
                            nc.tensor.matmul(
                                p3[:, :tn], w3[:, k, f * P:(f + 1) * P],
                                xt[:, k, :tn], start=(k == 0), stop=(k == KO1 - 1))
                        sl = spool.tile([P, NT], f32, tag="sl")
                        nc.scalar.activation(sl[:, :tn], p1[:, :tn], SILU)
                        nc.vector.tensor_mul(h[:, f, :tn], sl[:, :tn], p3[:, :tn])
                    for dch in range(KO1):
                        py = pspool.tile([P, NT], f32, tag="py")
                        for f in range(KO2):
                            nc.tensor.matmul(
                                py[:, :tn], w2[:, f, dch * P:(dch + 1) * P],
                                h[:, f, :tn], start=(f == 0), stop=(f == KO2 - 1))
                        yo = ypool.tile([P, NT], f32, tag="yo")
                        if scale_tile is not None:
                            nc.vector.tensor_mul(
                                yo[:, :tn], py[:, :tn], scale_tile[:, t0:t0 + tn])
                        else:
                            nc.vector.tensor_scalar_mul(
                                yo[:, :tn], py[:, :tn], 1.0 / 3.0)
                        nc.sync.dma_start(outr[:, dch, t0:t0 + tn], yo[:, :tn])

            ffn(xg, we1, we3, we2, ye, C, wb)
            ffn(xs, ws1, ws3, ws2, ys, TS, None)
    return nc


def kernel(x, Wg, Ws1, Ws3, Ws2, We1, We3, We2):
    global LAST_EXEC_NS
    x = np.asarray(x)
    xf = np.ascontiguousarray(x.reshape(-1, D_MODEL).astype(np.float32))
    T = xf.shape[0]
    assert T == T_TOKENS, f"kernel compiled for T={T_TOKENS}, got {T}"

    # ---- host routing (gate in fp64; matches the fp32 reference ranking) ----
    logits = xf.astype(np.float64) @ np.asarray(Wg, np.float64)
    gates = 1.0 / (1.0 + np.exp(-logits))
    order = np.argsort(-gates, axis=1, kind="stable")
    idx = order[:, :TOP_K]                                   # [T, 2]
    vals = np.take_along_axis(gates, idx, axis=1)
    w = vals / vals.sum(axis=1, keepdims=True)               # [T, 2]

    tok_lists = [np.where((idx == e).any(axis=1))[0] for e in range(N_ROUTED)]
    load = max(len(t) for t in tok_lists)
    C = max(P, ((load + P - 1) // P) * P)

    xf16 = xf.astype(_BF16)
    in_maps = []
    for e in range(N_ROUTED):
        tok = tok_lists[e]
        L = len(tok)
        xg = np.zeros((D_MODEL, C), _BF16)
        xg[:, :L] = xf16[tok].T
        sel = np.where(idx[tok, 0] == e, w[tok, 0], w[tok, 1])
        wv = np.zeros((C,), np.float32)
        wv[:L] = (sel / 3.0).astype(np.float32)
        in_maps.append({
            "xg": xg,
            "wgt": np.ascontiguousarray(np.broadcast_to(wv, (P, C))),
            "xs": np.ascontiguousarray(xf16[e * TS:(e + 1) * TS].T),
            "we1": np.ascontiguousarray(np.asarray(We1[e], np.float32).astype(_BF16)),
            "we3": np.ascontiguousarray(np.asarray(We3[e], np.float32).astype(_BF16)),
            "we2": np.ascontiguousarray(np.asarray(We2[e], np.float32).astype(_BF16)),
            "ws1": np.ascontiguousarray(np.asarray(Ws1, np.float32).astype(_BF16)),
            "ws3": np.ascontiguousarray(np.asarray(Ws3, np.float32).astype(_BF16)),
            "ws2": np.ascontiguousarray(np.asarray(Ws2, np.float32).astype(_BF16)),
        })

    nc = _build_program(C)
    profile = bool(int(os.environ.get("KERNEL_PROFILE", "0")))
    if profile:
        profile = _install_profiling_shims()
    res = run_bass_kernel_spmd(
        nc, in_maps, core_ids=list(range(N_CORES)), trace=profile)
    LAST_EXEC_NS = res.exec_time_ns

    out = np.zeros((T, D_MODEL), np.float32)
    for e in range(N_ROUTED):
        r = res.results[e]
        tok = tok_lists[e]
        out[tok] += r["ye"][:, :len(tok)].T
        out[e * TS:(e + 1) * TS] += r["ys"].T
    return out.reshape(x.shape)


# revision 16
# speedup vs baseline: 1.1810x; 1.1810x over previous
"""DeepSeek-style MoE block on 8 Trainium2 NeuronCores.

Sharding strategy (expert-parallel, per the problem's sharding hint):
  - The 8 routed experts are sharded one-per-core: core e holds We1[e]/We3[e]/We2[e].
  - Token dispatch (gate -> top-2 -> gather per expert) is computed on host;
    each core receives its expert's gathered token batch, padded to a common
    capacity C, in transposed [d_model, C] layout so no on-device transposes
    are needed (activations stay [feature, token] throughout).
  - The shared expert is data-parallel: core e processes tokens [e*512,(e+1)*512).
  - Matmuls run in bf16 with fp32 PSUM accumulation; outputs are fp32.
  - Host scatter-adds the per-expert outputs (already scaled by the normalized
    gate weight / 3 on device) and the shared outputs (scaled by 1/3 on device).
"""

import os

import numpy as np
import ml_dtypes

import concourse.bass as bass
import concourse.mybir as mybir
import concourse.tile as tile_mod
from concourse.bass_utils import run_bass_kernel_spmd
from concourse.vector_clock import ScopedClock

_BF16 = ml_dtypes.bfloat16
P = 128
D_MODEL = 1024
FF_DIM = 2048
N_ROUTED = 8
TOP_K = 2
N_CORES = 8
T_TOKENS = 4096           # 2 * 2048
TS = T_TOKENS // N_CORES  # shared-expert tokens per core
KO1 = D_MODEL // P        # 8  k-chunks for d_model contraction
KO2 = FF_DIM // P         # 16 k-chunks for ff contraction
NT = 512                  # token tile (matmul moving free dim)

LAST_EXEC_NS = None       # set after each kernel() call when profiling


def _split_drain_and_barrier(self, tick_clock, wait_clock):
    """Tile's kernel-tail drain carries one sem-wait per touched engine/queue;
    this walrus build rejects >2 sync waits per instruction. Split the waits
    into single-wait sync nops ahead of the drain (semantically identical:
    the SP stream blocks on each wait in sequence before drain+barrier)."""
    nc = self.nc
    probe = nc.sync.nop(nofuse=True)
    wait_clock.add_sem_waits(probe.ins, ScopedClock({None: tick_clock.global_clock}))
    waits = list(probe.ins.sync_info.on_wait)
    if len(waits) > 1:
        probe.ins.sync_info.on_wait[:] = waits[:1]
        SyncInfo = type(probe.ins.sync_info)
        for w in waits[1:]:
            n2 = nc.sync.nop(nofuse=True)
            n2.ins.sync_info = SyncInfo(on_update=[], on_wait=[w])
    nc.sync.drain()
    nc.all_engine_barrier()
    popped = nc._tile_sem_poison_stack.pop()
    assert popped is self._sem_poison
    nc.clear_and_free_semaphores(list(self.sems.allocated().values()))
    nc.all_engine_barrier()


_MAX_WAITS = 1  # this walrus build rejects multiple sync waits on one instruction


def _split_body_waits(self, postordered_blocks):
    """Before lowering the scheduled instruction lists into basic blocks,
    hoist excess sem-waits (>_MAX_WAITS) of any instruction onto same-engine
    nops inserted immediately before it. Same-engine order is preserved, so
    semantics are identical."""
    nc = self.nc
    for insts in postordered_blocks.values():
        needs_fix = any(
            getattr(ins, "sync_info", None) is not None
            and len(ins.sync_info.on_wait) > _MAX_WAITS
            and getattr(ins, "engine", None) is not None
            for ins in insts
        )
        if not needs_fix:
            continue
        out = []
        for ins in insts:
            si = getattr(ins, "sync_info", None)
            if (si is not None and len(si.on_wait) > _MAX_WAITS
                    and getattr(ins, "engine", None) is not None):
                waits = list(si.on_wait)
                excess, keep = waits[:-_MAX_WAITS], waits[-_MAX_WAITS:]
                si.on_wait[:] = keep
                for i in range(0, len(excess), _MAX_WAITS):
                    out.append(mybir.InstNoOp(
                        name=nc.get_next_instruction_name(),
                        engine=ins.engine,
                        bass_nofuse=True,
                        sync_info=mybir.SyncInfo(
                            on_wait=list(excess[i:i + _MAX_WAITS]), on_update=[]),
                    ))
            out.append(ins)
        insts[:] = out
    return tile_mod.TileContext._orig_lower_ordered_insts(self, postordered_blocks)


def _install_drain_fix():
    if getattr(tile_mod.TileContext, "_drain_fix_installed", False):
        return
    tile_mod.TileContext._drain_and_barrier = _split_drain_and_barrier
    tile_mod.TileContext._orig_lower_ordered_insts = (
        tile_mod.TileContext._lower_ordered_insts)
    tile_mod.TileContext._lower_ordered_insts = _split_body_waits
    tile_mod.TileContext._drain_fix_installed = True


def _install_profiling_shims():
    """Best-effort NTFF profiling under axon: the container's `antenv` lacks
    `axon_hooks`, so build it and register the ctypes hook from trn_agent_boot.
    Also neuter the S3 artifact upload (no credentials here)."""
    import sys
    import types

    import concourse.bass_utils as bu
    bu.upload_artifacts = lambda tmpdir: tmpdir

    try:
        import antenv.axon_hooks  # noqa: F401
        return True
    except ImportError:
        pass
    try:
        from trn_agent_boot.trn_boot import _ntff_profile_via_ctypes
        hook = _ntff_profile_via_ctypes("/opt/axon/libaxon_pjrt.so")
    except Exception:
        return False
    if hook is None:
        return False
    m = types.ModuleType("antenv.axon_hooks")
    _state = {"h": hook}
    m.get_axon_ntff_profile_hook = lambda: _state["h"]
    m.set_axon_ntff_profile_hook = lambda h: _state.__setitem__("h", h)
    sys.modules["antenv.axon_hooks"] = m
    import antenv
    antenv.axon_hooks = m
    return True


def _token_tiles(c):
    """Split c tokens into near-equal tiles of <=NT (keeps every matmul's
    moving free dim large enough that LDWEIGHTS stays hidden)."""
    n = -(-c // NT)
    base, rem = divmod(c, n)
    tiles, t0 = [], 0
    for i in range(n):
        tn = base + (1 if i < rem else 0)
        tiles.append((t0, tn))
        t0 += tn
    return tiles


def _block_weights(w):
    """[K, M] -> [M//P, P, K//P, P] where [mo, p, ko, m] = w[ko*P+p, mo*P+m].
    Each [mo] block is one contiguous 256KB DMA and one dependency unit, so
    matmul chains can start as soon as their own column block lands."""
    K, M = w.shape
    return np.ascontiguousarray(
        np.asarray(w, np.float32).astype(_BF16)
        .reshape(K // P, P, M // P, P).transpose(2, 1, 0, 3))


_PROG_CACHE = {}


def _get_program(C):
    if C not in _PROG_CACHE:
        _PROG_CACHE[C] = _build_program(C)
    return _PROG_CACHE[C]


def _build_program(C):
    """One SPMD program: expert FFN over C gathered tokens + shared FFN over TS
    tokens. All activations live in [feature, token] (transposed) layout."""
    _install_drain_fix()
    nc = bass.Bass("TRN2")
    f32, bf16 = mybir.dt.float32, mybir.dt.bfloat16
    SILU = mybir.ActivationFunctionType.Silu

    xg = nc.dram_tensor("xg", [D_MODEL, C], bf16, kind="ExternalInput")
    wgt = nc.dram_tensor("wgt", [P, C], f32, kind="ExternalInput")
    xs = nc.dram_tensor("xs", [D_MODEL, TS], bf16, kind="ExternalInput")
    # weights arrive pre-blocked from host: [m_blk, P, k_blk, P]
    we1 = nc.dram_tensor("we1", [KO2, P, KO1, P], bf16, kind="ExternalInput")
    we3 = nc.dram_tensor("we3", [KO2, P, KO1, P], bf16, kind="ExternalInput")
    we2 = nc.dram_tensor("we2", [KO1, P, KO2, P], bf16, kind="ExternalInput")
    ws1 = nc.dram_tensor("ws1", [KO2, P, KO1, P], bf16, kind="ExternalInput")
    ws3 = nc.dram_tensor("ws3", [KO2, P, KO1, P], bf16, kind="ExternalInput")
    ws2 = nc.dram_tensor("ws2", [KO1, P, KO2, P], bf16, kind="ExternalInput")
    ye = nc.dram_tensor("ye", [D_MODEL, C], f32, kind="ExternalOutput")
    ys = nc.dram_tensor("ys", [D_MODEL, TS], f32, kind="ExternalOutput")

    with tile_mod.TileContext(nc) as tc:
        with (
            tc.tile_pool(name="wpool", bufs=KO2) as w13pool,
            tc.tile_pool(name="w2pool", bufs=2 * KO1) as w2pool,
            tc.tile_pool(name="xpool", bufs=2) as xpool,
            tc.tile_pool(name="hpool", bufs=1) as hpool,
            tc.tile_pool(name="spool", bufs=3) as spool,
            tc.tile_pool(name="ypool", bufs=3) as ypool,
            tc.tile_pool(name="cpool", bufs=1) as cpool,
            tc.tile_pool(name="pspool", bufs=2, space="PSUM") as pspool,
        ):
            wb = cpool.tile([P, C], f32, tag="wb")

            def ffn(xT, w1d, w3d, w2d, outd, Ct, scale_tile, first=False):
                xr = xT[:, :].rearrange("(ko p) t -> p ko t", p=P)
                outr = outd[:, :].rearrange("(ko p) t -> p ko t", p=P)
                tiles = _token_tiles(Ct)
                # first token tile's x gates the very first matmul chain:
                # its DMA goes ahead of every weight-block load
                t0_0, tn_0 = tiles[0]
                xt0 = xpool.tile([P, KO1, NT], bf16, tag="xt")
                nc.sync.dma_start(xt0[:, :, :tn_0], xr[:, :, t0_0:t0_0 + tn_0])
                # per-column-block weight tiles; DMA order matches the f-chain
                # consumption order so TensorE starts after the first block
                w1b, w3b = [], []
                for f in range(KO2):
                    t1 = w13pool.tile([P, KO1, P], bf16, tag="w1f")
                    nc.sync.dma_start(t1[:], w1d[f])
                    w1b.append(t1)
                    t3 = w13pool.tile([P, KO1, P], bf16, tag="w3f")
                    nc.sync.dma_start(t3[:], w3d[f])
                    w3b.append(t3)
                w2b = []
                for ti, (t0, tn) in enumerate(tiles):
                    if ti == 0:
                        xt = xt0
                    else:
                        xt = xpool.tile([P, KO1, NT], bf16, tag="xt")
                        nc.sync.dma_start(xt[:, :, :tn], xr[:, :, t0:t0 + tn])
                    h = hpool.tile([P, KO2, NT], bf16, tag="h")
                    for f in range(KO2):
                        p1 = pspool.tile([P, NT], f32, tag="p1")
                        p3 = pspool.tile([P, NT], f32, tag="p3")
                        for k in range(KO1):
                            nc.tensor.matmul(
                                p1[:, :tn], w1b[f][:, k, :],
                                xt[:, k, :tn], start=(k == 0), stop=(k == KO1 - 1))
                        for k in range(KO1):
                            nc.tensor.matmul(
                                p3[:, :tn], w3b[f][:, k, :],
                                xt[:, k, :tn], start=(k == 0), stop=(k == KO1 - 1))
                        sl = spool.tile([P, NT], f32, tag="sl")
                        nc.scalar.activation(sl[:, :tn], p1[:, :tn], SILU)
                        nc.vector.tensor_mul(h[:, f, :tn], sl[:, :tn], p3[:, :tn])
                    if ti == 0:
                        # stage-2 weights (and the gate-scale broadcast) are
                        # first needed now; keeping their DMAs behind the
                        # stage-1-critical loads preserves startup latency
                        if first:
                            nc.sync.dma_start(wb[:], wgt[:, :])
                        for dch in range(KO1):
                            t2 = w2pool.tile([P, KO2, P], bf16, tag="w2d")
                            nc.sync.dma_start(t2[:], w2d[dch])
                            w2b.append(t2)
                    for dch in range(KO1):
                        py = pspool.tile([P, NT], f32, tag="py")
                        for f in range(KO2):
                            nc.tensor.matmul(
                                py[:, :tn], w2b[dch][:, f, :],
                                h[:, f, :tn], start=(f == 0), stop=(f == KO2 - 1))
                        yo = ypool.tile([P, NT], f32, tag="yo")
                        if scale_tile is not None:
                            nc.vector.tensor_mul(
                                yo[:, :tn], py[:, :tn], scale_tile[:, t0:t0 + tn])
                        else:
                            nc.vector.tensor_scalar_mul(
                                yo[:, :tn], py[:, :tn], 1.0 / 3.0)
                        nc.sync.dma_start(outr[:, dch, t0:t0 + tn], yo[:, :tn])

            ffn(xg, we1, we3, we2, ye, C, wb, first=True)
            ffn(xs, ws1, ws3, ws2, ys, TS, None)
    return nc


def kernel(x, Wg, Ws1, Ws3, Ws2, We1, We3, We2):
    global LAST_EXEC_NS
    x = np.asarray(x)
    xf = np.ascontiguousarray(x.reshape(-1, D_MODEL).astype(np.float32))
    T = xf.shape[0]
    assert T == T_TOKENS, f"kernel compiled for T={T_TOKENS}, got {T}"

    # ---- host routing (gate in fp64; matches the fp32 reference ranking) ----
    logits = xf.astype(np.float64) @ np.asarray(Wg, np.float64)
    gates = 1.0 / (1.0 + np.exp(-logits))
    order = np.argsort(-gates, axis=1, kind="stable")
    idx = order[:, :TOP_K]                                   # [T, 2]
    vals = np.take_along_axis(gates, idx, axis=1)
    w = vals / vals.sum(axis=1, keepdims=True)               # [T, 2]

    tok_lists = [np.where((idx == e).any(axis=1))[0] for e in range(N_ROUTED)]
    load = max(len(t) for t in tok_lists)
    C = max(P, ((load + 15) // 16) * 16)

    xf16 = xf.astype(_BF16)
    ws1_b = _block_weights(Ws1)
    ws3_b = _block_weights(Ws3)
    ws2_b = _block_weights(Ws2)
    in_maps = []
    for e in range(N_ROUTED):
        tok = tok_lists[e]
        L = len(tok)
        xg = np.zeros((D_MODEL, C), _BF16)
        xg[:, :L] = xf16[tok].T
        sel = np.where(idx[tok, 0] == e, w[tok, 0], w[tok, 1])
        wv = np.zeros((C,), np.float32)
        wv[:L] = (sel / 3.0).astype(np.float32)
        in_maps.append({
            "xg": xg,
            "wgt": np.ascontiguousarray(np.broadcast_to(wv, (P, C))),
            "xs": np.ascontiguousarray(xf16[e * TS:(e + 1) * TS].T),
            "we1": _block_weights(We1[e]),
            "we3": _block_weights(We3[e]),
            "we2": _block_weights(We2[e]),
            "ws1": ws1_b, "ws3": ws3_b, "ws2": ws2_b,
        })

    nc = _get_program(C)
    profile = bool(int(os.environ.get("KERNEL_PROFILE", "0")))
    if profile:
        profile = _install_profiling_shims()
    try:
        res = run_bass_kernel_spmd(
            nc, in_maps, core_ids=list(range(N_CORES)), trace=profile,
            tmpdir=os.environ.get("KERNEL_TRACE_DIR") or None)
    except Exception:
        # transient device hiccups (e.g. NRT_EXEC_UNIT_UNRECOVERABLE) recover
        # on the next dispatch; retry once without profiling
        res = run_bass_kernel_spmd(
            nc, in_maps, core_ids=list(range(N_CORES)), trace=False)
    LAST_EXEC_NS = res.exec_time_ns
    globals()["LAST_RESULTS"] = res

    out = np.zeros((T, D_MODEL), np.float32)
    for e in range(N_ROUTED):
        r = res.results[e]
        tok = tok_lists[e]
        out[tok] += r["ye"][:, :len(tok)].T
        out[e * TS:(e + 1) * TS] += r["ys"].T
    return out.reshape(x.shape)
